# revision 3
# baseline (speedup 1.0000x reference)
"""Multi-head attention (bs=2, seq=2048, d_model=1024, 16 heads) on 8 NeuronCores.

Sharding: core = b*4 + g  (b = batch 0..1, g = head-group 0..3, 4 heads each).
Each core computes, for its batch b and head slice s256 = [256g, 256g+256):
  qhT [256, 2048] = (0.125*W_q[s256]) @ q[b].T      (scores scale folded into W_q)
  khT [256, 2048] = W_k[s256] @ k[b].T
  vh  [2048, 256] = v[b] @ W_v[s256].T              (+ ones column per head)
  per head: S^T = khT.T-slices @ qhT  -> exp -> P^T (bf16)
            attnU^T[65, sq] = vh_aug.T @ P^T        (row 64 = softmax sums)
            normalize via PE-transposed reciprocal sums
  out_partial [2048, 1024] = attnN @ W_o[:, s256].T   (f32)
Host sums the 4 partials per batch and adds b_o.
"""

import sys

sys.path.insert(0, "/opt/trn_rl_repo")

import numpy as np
import ml_dtypes

import concourse.bass as bass
import concourse.mybir as mybir
import concourse.tile as tile
from concourse import bacc
from concourse.bass_utils import run_bass_kernel_spmd
from concourse.masks import make_identity

BF16 = ml_dtypes.bfloat16
F32 = mybir.dt.float32
BF = mybir.dt.bfloat16

SEQ = 2048
DM = 1024
NH_CORE = 4          # heads per core
DK = 64
DSL = 256            # head dims per core
NT = SEQ // 128      # 16 seq tiles
NC4 = 4              # seq chunks of 512

_cache = {}


def _build():
    nc = bacc.Bacc(None, target_bir_lowering=False, debug=False)
    with tile.TileContext(nc) as tc:
        with tc.tile_pool(name="dram", bufs=1, space="DRAM") as dram:
            qT_d = dram.tile([128, 8, SEQ], BF, kind="ExternalInput", tag="qT")
            kT_d = dram.tile([128, 8, SEQ], BF, kind="ExternalInput", tag="kT")
            vT_d = dram.tile([128, 8, SEQ], BF, kind="ExternalInput", tag="vT")
            wq_d = dram.tile([128, 8, DSL], BF, kind="ExternalInput", tag="wq")
            wk_d = dram.tile([128, 8, DSL], BF, kind="ExternalInput", tag="wk")
            wv_d = dram.tile([128, 8, DSL], BF, kind="ExternalInput", tag="wv")
            wo_d = dram.tile([128, 2, DM], BF, kind="ExternalInput", tag="wo")
            out_d = dram.tile([SEQ, DM], F32, kind="ExternalOutput", tag="out")

            with tc.tile_pool(name="const", bufs=1) as cp:
                wq_sb = cp.tile([128, 8, DSL], BF, tag="cwq")
                wk_sb = cp.tile([128, 8, DSL], BF, tag="cwk")
                wv_sb = cp.tile([128, 8, DSL], BF, tag="cwv")
                wo_sb = cp.tile([128, 2, DM], BF, tag="cwo")
                ident = cp.tile([128, 128], F32, tag="cid")
                ones = cp.tile([1, 64], F32, tag="cones")
                nc.sync.dma_start(wq_sb[:], wq_d[:])
                nc.sync.dma_start(wk_sb[:], wk_d[:])
                nc.sync.dma_start(wv_sb[:], wv_d[:])
                nc.sync.dma_start(wo_sb[:], wo_d[:])
                make_identity(nc, ident[:])
                nc.gpsimd.memset(ones[:], 1.0)

                with tc.tile_pool(name="persist", bufs=1) as pp:
                    qh_sb = pp.tile([128, 2, SEQ], BF, tag="qh")
                    kh_sb = pp.tile([128, 2, SEQ], BF, tag="kh")
                    vh_sb = pp.tile([128, NT, 260], BF, tag="vh")
                    att_sb = pp.tile([128, 2, SEQ], BF, tag="att")
                    # ones columns of vh (col 64 of each 65-wide head block)
                    vh_ones = vh_sb[:].rearrange(
                        "p m (h x) -> p m h x", h=4
                    )[:, :, :, 64:65]
                    nc.vector.memset(vh_ones, 1.0)

                    # ---------------- projections ----------------
                    with (
                        tc.tile_pool(name="io", bufs=1) as io,
                        tc.tile_pool(name="pjp", bufs=1, space="PSUM") as pj,
                    ):
                        qt = [io.tile([128, SEQ], BF, tag="qt", bufs=8, name=f"qt{j}") for j in range(8)]
                        kt = [io.tile([128, SEQ], BF, tag="kt", bufs=8, name=f"kt{j}") for j in range(8)]
                        vt = [io.tile([128, SEQ], BF, tag="vt", bufs=8, name=f"vt{j}") for j in range(8)]
                        for j in range(8):
                            nc.sync.dma_start(qt[j][:], qT_d[:, j, :])
                            nc.sync.dma_start(kt[j][:], kT_d[:, j, :])
                            nc.sync.dma_start(vt[j][:], vT_d[:, j, :])

                        for (w_sb, x_t, o_sb) in ((wq_sb, qt, qh_sb), (wk_sb, kt, kh_sb)):
                            for m in range(2):
                                for n in range(NC4):
                                    ps = pj.tile([128, 512], F32, tag="pj", bufs=4)
                                    for j in range(8):
                                        nc.tensor.matmul(
                                            ps[:],
                                            w_sb[:, j, m * 128 : (m + 1) * 128],
                                            x_t[j][:, n * 512 : (n + 1) * 512],
                                            start=(j == 0),
                                            stop=(j == 7),
                                        )
                                    nc.vector.tensor_copy(
                                        o_sb[:, m, n * 512 : (n + 1) * 512], ps[:]
                                    )
                        for m in range(NT):
                            ps = pj.tile([128, 512], F32, tag="pj", bufs=4)
                            for j in range(8):
                                nc.tensor.matmul(
                                    ps[:, 0:DSL],
                                    vt[j][:, m * 128 : (m + 1) * 128],
                                    wv_sb[:, j, :],
                                    start=(j == 0),
                                    stop=(j == 7),
                                )
                            nc.vector.tensor_copy(
                                vh_sb[:, m, :].rearrange("p (h x) -> p h x", h=4)[
                                    :, :, 0:64
                                ],
                                ps[:, 0:DSL].rearrange("p (h x) -> p h x", h=4),
                            )

                    # ---------------- attention ----------------
                    with (
                        tc.tile_pool(name="asb", bufs=1) as ap,
                        tc.tile_pool(name="aps", bufs=1, space="PSUM") as aps,
                    ):
                        out_sb_pool = ap
                        for h in range(NH_CORE):
                            t, hh = h // 2, h % 2
                            p0 = 64 * hh
                            # S^T + exp, per s_k tile
                            pts = []
                            for m in range(NT):
                                s_ps = aps.tile([128, SEQ], F32, tag="s", bufs=1)
                                for n in range(NC4):
                                    nc.tensor.matmul(
                                        s_ps[:, n * 512 : (n + 1) * 512],
                                        kh_sb[p0 : p0 + 64, t, m * 128 : (m + 1) * 128],
                                        qh_sb[p0 : p0 + 64, t, n * 512 : (n + 1) * 512],
                                        start=True,
                                        stop=True,
                                    )
                                pt = ap.tile([128, SEQ], BF, tag="pt", bufs=18)
                                nc.scalar.activation(
                                    pt[:], s_ps[:], mybir.ActivationFunctionType.Exp
                                )
                                pts.append(pt)
                            # attnU^T (rows 0-63) + sums (row 64)
                            u_sb = ap.tile([65, SEQ], F32, tag="u", bufs=2)
                            for n in range(NC4):
                                av = aps.tile([65, 512], F32, tag="av", bufs=2)
                                for m in range(NT):
                                    nc.tensor.matmul(
                                        av[:],
                                        vh_sb[:, m, 65 * h : 65 * h + 65],
                                        pts[m][:, n * 512 : (n + 1) * 512],
                                        start=(m == 0),
                                        stop=(m == NT - 1),
                                    )
                                nc.vector.tensor_copy(u_sb[:, n * 512 : (n + 1) * 512], av[:])
                            # reciprocal of sums via PE transposes
                            sT = aps.tile([128, NT], F32, tag="nrm", bufs=2)
                            for m in range(NT):
                                nc.tensor.transpose(
                                    sT[:, m : m + 1],
                                    u_sb[64:65, m * 128 : (m + 1) * 128],
                                    ident[64:65, 64:65],
                                )
                            rT = ap.tile([128, NT], F32, tag="rT", bufs=2)
                            nc.vector.reciprocal(rT[:], sT[:])
                            rs = ap.tile([1, SEQ], F32, tag="rs", bufs=2)
                            stage = (
                                ap.tile([64, SEQ], BF, tag="stg", bufs=2, name="stage") if hh else None
                            )
                            for n in range(NC4):
                                row = aps.tile([1, 512], F32, tag="nrm", bufs=2)
                                for i in range(4):
                                    nc.tensor.transpose(
                                        row[0:1, i * 128 : (i + 1) * 128],
                                        rT[:, 4 * n + i : 4 * n + i + 1],
                                        ident[:, 0:128],
                                    )
                                nc.vector.tensor_copy(rs[0:1, n * 512 : (n + 1) * 512], row[:])
                                bc = aps.tile([64, 512], F32, tag="nrm", bufs=2)
                                nc.tensor.matmul(
                                    bc[:],
                                    ones[0:1, :],
                                    rs[0:1, n * 512 : (n + 1) * 512],
                                    start=True,
                                    stop=True,
                                )
                                tgt = (
                                    stage[:, n * 512 : (n + 1) * 512]
                                    if hh
                                    else att_sb[0:64, t, n * 512 : (n + 1) * 512]
                                )
                                nc.vector.tensor_mul(
                                    tgt, u_sb[0:64, n * 512 : (n + 1) * 512], bc[:]
                                )
                            if hh:
                                nc.sync.dma_start(att_sb[64:128, t, :], stage[:])

                        # ---------------- output projection ----------------
                        for s in range(NT):
                            for c in range(2):
                                op = aps.tile([128, 512], F32, tag="av", bufs=2)
                                for kt2 in range(2):
                                    nc.tensor.matmul(
                                        op[:],
                                        att_sb[:, kt2, s * 128 : (s + 1) * 128],
                                        wo_sb[:, kt2, c * 512 : (c + 1) * 512],
                                        start=(kt2 == 0),
                                        stop=(kt2 == 1),
                                    )
                                ot = out_sb_pool.tile([128, 512], F32, tag="o", bufs=3)
                                nc.any.tensor_copy(ot[:], op[:])
                                nc.sync.dma_start(
                                    out_d[s * 128 : (s + 1) * 128, c * 512 : (c + 1) * 512],
                                    ot[:],
                                )
    nc.compile()
    names = dict(
        qT=qT_d.name, kT=kT_d.name, vT=vT_d.name,
        wq=wq_d.name, wk=wk_d.name, wv=wv_d.name, wo=wo_d.name, out=out_d.name,
    )
    return nc, names


def _dev_layout_x(x):
    # [seq, dm] f32 -> transposed [dm, seq] -> [128, 8, seq] bf16
    xt = np.ascontiguousarray(x.T).astype(BF16)
    return np.ascontiguousarray(xt.reshape(8, 128, SEQ).swapaxes(0, 1))


def _dev_layout_w(w):
    # [256, dm] slice -> W.T [dm, 256] -> [128, 8, 256] bf16
    wt = np.ascontiguousarray(w.T).astype(BF16)
    return np.ascontiguousarray(wt.reshape(8, 128, DSL).swapaxes(0, 1))


def kernel(q, k, v, W_q, b_q, W_k, b_k, W_v, b_v, W_o, b_o, trace=False):
    if "nc" not in _cache:
        _cache["nc"], _cache["names"] = _build()
    nc, names = _cache["nc"], _cache["names"]

    q, k, v = np.asarray(q), np.asarray(k), np.asarray(v)
    in_maps = []
    for core in range(8):
        b, g = core // 4, core % 4
        s256 = slice(256 * g, 256 * (g + 1))
        wo_slice = np.ascontiguousarray(np.asarray(W_o)[:, s256].T).astype(BF16)
        in_maps.append({
            names["qT"]: _dev_layout_x(q[b]),
            names["kT"]: _dev_layout_x(k[b]),
            names["vT"]: _dev_layout_x(v[b]),
            names["wq"]: _dev_layout_w(np.asarray(W_q)[s256] * 0.125),
            names["wk"]: _dev_layout_w(np.asarray(W_k)[s256]),
            names["wv"]: _dev_layout_w(np.asarray(W_v)[s256]),
            names["wo"]: np.ascontiguousarray(
                wo_slice.reshape(2, 128, DM).swapaxes(0, 1)
            ),
        })

    res = run_bass_kernel_spmd(nc, in_maps, core_ids=list(range(8)), trace=trace)
    out = np.zeros((2, SEQ, DM), np.float32)
    for core in range(8):
        out[core // 4] += res.results[core][names["out"]]
    out += np.asarray(b_o)[None, None, :].astype(np.float32)
    _cache["last_res"] = res
    return out


# revision 11
# speedup vs baseline: 40172455.8473x; 40172455.8473x over previous
"""Multi-head attention (bs=2, seq=2048, d_model=1024, 16 heads) on 8 NeuronCores.

Sharding: core = b*4 + g  (b = batch 0..1, g = head-group 0..3, 4 heads each).
Per core, for batch b and head slice s256 = [256g, 256g+256):
  qhT [256, 2048] = (0.125*W_q[s256]) @ q[b].T      (scores scale folded into W_q)
  khT [256, 2048] = W_k[s256] @ k[b].T
  vh  [2048, 260] = v[b] @ W_v[s256].T              (+ ones column per head)
  per head: S^T = khT-slice.T @ qhT -> exp -> P^T (bf16)
            attnU^T[65, sq] = vh_aug.T @ P^T        (row 64 = softmax sums)
            normalize with PE-transposed reciprocal sums
  out_partial [2048, 1024] = attnN @ W_o[:, s256].T   (f32)
Host sums the 4 partials per batch and adds b_o.
Head pairs (2t, 2t+1) interleave their K=64 S^T matmuls on PE row groups
0-1 / 2-3 so the systolic array runs both concurrently.
"""

import sys

sys.path.insert(0, "/opt/trn_rl_repo")

import numpy as np
import ml_dtypes

import concourse.bass as bass
import concourse.mybir as mybir
import concourse.tile as tile
from concourse import bacc
from concourse.bass_utils import run_bass_kernel_spmd
from concourse.masks import make_identity

BF16 = ml_dtypes.bfloat16
F32 = mybir.dt.float32
BF = mybir.dt.bfloat16

SEQ = 2048
DM = 1024
DSL = 256            # head dims per core
NT = SEQ // 128      # 16 seq tiles
NC4 = 4              # seq chunks of 512

_cache = {}


def _build(reps=1):
    nc = bacc.Bacc(None, target_bir_lowering=False, debug=False)
    with tile.TileContext(nc) as tc:
        with tc.tile_pool(name="dram", bufs=1, space="DRAM") as dram:
            qT_d = dram.tile([128, 8, SEQ], BF, kind="ExternalInput", tag="qT")
            kT_d = dram.tile([128, 8, SEQ], BF, kind="ExternalInput", tag="kT")
            vT_d = dram.tile([128, 8, SEQ], BF, kind="ExternalInput", tag="vT")
            wq_d = dram.tile([128, 8, DSL], BF, kind="ExternalInput", tag="wq")
            wk_d = dram.tile([128, 8, DSL], BF, kind="ExternalInput", tag="wk")
            wv_d = dram.tile([128, 8, DSL], BF, kind="ExternalInput", tag="wv")
            wo_d = dram.tile([128, 2, DM], BF, kind="ExternalInput", tag="wo")
            out_d = dram.tile([SEQ, DM], F32, kind="ExternalOutput", tag="out")

            with tc.tile_pool(name="const", bufs=1) as cp:
                wo_sb = cp.tile([128, 2, DM], BF, tag="cwo")
                ident = cp.tile([128, 128], F32, tag="cid")
                ones = cp.tile([1, 64], F32, tag="cones")
                nc.sync.dma_start(wo_sb[:], wo_d[:])
                make_identity(nc, ident[:])
                nc.gpsimd.memset(ones[:], 1.0)

                with tc.tile_pool(name="persist", bufs=1) as pp:
                    qh_sb = pp.tile([128, 2, SEQ], BF, tag="qh")
                    kh_sb = pp.tile([128, 2, SEQ], BF, tag="kh")
                    vh_sb = pp.tile([128, NT, 260], BF, tag="vh")
                    vh_ones = vh_sb[:].rearrange(
                        "p m (h x) -> p m h x", h=4
                    )[:, :, :, 64:65]
                    nc.vector.memset(vh_ones, 1.0)

                    for _rep in range(reps):
                        with (
                            tc.tile_pool(name="aps", bufs=1, space="PSUM") as aps,
                            tc.tile_pool(name="ptp", bufs=1) as ptp,
                        ):
                            # ---------------- q/k projections ----------------
                            with tc.tile_pool(name="ioqk", bufs=1) as io:
                                wq_sb = io.tile([128, 8, DSL], BF, tag="cwq")
                                wk_sb = io.tile([128, 8, DSL], BF, tag="cwk")
                                nc.sync.dma_start(wq_sb[:], wq_d[:])
                                nc.sync.dma_start(wk_sb[:], wk_d[:])
                                qt = [io.tile([128, SEQ], BF, tag="qt", bufs=8, name=f"qt{j}") for j in range(8)]
                                kt = [io.tile([128, SEQ], BF, tag="kt", bufs=8, name=f"kt{j}") for j in range(8)]
                                for j in range(8):
                                    nc.sync.dma_start(qt[j][:], qT_d[:, j, :])
                                for j in range(8):
                                    nc.sync.dma_start(kt[j][:], kT_d[:, j, :])
                                for m in range(2):
                                    for (w_sb, x_t, o_sb) in ((wq_sb, qt, qh_sb), (wk_sb, kt, kh_sb)):
                                        for n in range(NC4):
                                            ps = aps.tile([128, 512], F32, tag="av", bufs=4, name=f"pj{m}{n}")
                                            for j in range(8):
                                                nc.tensor.matmul(
                                                    ps[:],
                                                    w_sb[:, j, m * 128 : (m + 1) * 128],
                                                    x_t[j][:, n * 512 : (n + 1) * 512],
                                                    start=(j == 0),
                                                    stop=(j == 7),
                                                )
                                            nc.vector.tensor_copy(
                                                o_sb[:, m, n * 512 : (n + 1) * 512], ps[:]
                                            )

                            with (
                                tc.tile_pool(name="iov", bufs=1) as iov,
                                tc.tile_pool(name="asb", bufs=1) as ap,
                            ):
                                att_sb = ap.tile([128, 2, SEQ], BF, tag="att")
                                wv_sb = iov.tile([128, 8, DSL], BF, tag="cwv")
                                nc.sync.dma_start(wv_sb[:], wv_d[:])
                                vt = [iov.tile([128, SEQ], BF, tag="vt", bufs=8, name=f"vt{j}") for j in range(8)]
                                for j in range(8):
                                    nc.sync.dma_start(vt[j][:], vT_d[:, j, :])

                                pts = [[] for _ in range(4)]
                                avs = {}
                                u_saved = {}

                                def s_step(h, m):
                                    t, p0 = h // 2, 64 * (h % 2)
                                    pt = ptp.tile([128, SEQ], BF, tag="pt", bufs=18,
                                                  name=f"pt{h}_{m}")
                                    for c in range(2):
                                        s_ps = aps.tile([128, 1024], F32, tag="s", bufs=2,
                                                        name=f"s{h}_{m}{c}")
                                        for n in range(2):
                                            nn = 2 * c + n
                                            nc.tensor.matmul(
                                                s_ps[:, n * 512 : (n + 1) * 512],
                                                kh_sb[p0 : p0 + 64, t, m * 128 : (m + 1) * 128],
                                                qh_sb[p0 : p0 + 64, t, nn * 512 : (nn + 1) * 512],
                                                start=True,
                                                stop=True,
                                            )
                                        nc.scalar.activation(
                                            pt[:, c * 1024 : (c + 1) * 1024],
                                            s_ps[:],
                                            mybir.ActivationFunctionType.Exp,
                                        )
                                    pts[h].append(pt)

                                def av_step(h, m):
                                    for n in range(NC4):
                                        nc.tensor.matmul(
                                            avs[h][n][0:65, :],
                                            vh_sb[:, m, 65 * h : 65 * h + 65],
                                            pts[h][m][:, n * 512 : (n + 1) * 512],
                                            start=(m == 0),
                                            stop=(m == NT - 1),
                                        )

                                def ucopy(h):
                                    u_sb = ap.tile([64, SEQ], BF, tag="u", bufs=3, name=f"u{h}")
                                    scs = []
                                    for n in range(NC4):
                                        nc.vector.tensor_copy(
                                            u_sb[:, n * 512 : (n + 1) * 512], avs[h][n][0:64, :]
                                        )
                                        sc = ap.tile([65, 512], F32, tag="sc", bufs=6, name=f"sc{h}{n}")
                                        nc.vector.tensor_copy(sc[64:65, :], avs[h][n][64:65, :])
                                        scs.append(sc)
                                    u_saved[h] = (u_sb, scs)

                                def normrest(h):
                                    t, hh = h // 2, h % 2
                                    u_sb, scs = u_saved[h]
                                    sT = aps.tile([128, NT], F32, tag="s", bufs=2, name=f"sT{h}")
                                    for m in range(NT):
                                        nc.tensor.transpose(
                                            sT[:, m : m + 1],
                                            scs[m // 4][64:65, (m % 4) * 128 : (m % 4 + 1) * 128],
                                            ident[64:65, 64:65],
                                        )
                                    rT = ap.tile([128, NT], F32, tag="rT", bufs=2, name=f"rT{h}")
                                    nc.vector.reciprocal(rT[:], sT[:])
                                    stage = (
                                        ap.tile([64, SEQ], BF, tag="u", bufs=3, name=f"stg{h}")
                                        if hh
                                        else None
                                    )
                                    for n in range(NC4):
                                        row = aps.tile([1, 512], F32, tag="s", bufs=2, name=f"row{h}{n}")
                                        for i in range(4):
                                            nc.tensor.transpose(
                                                row[0:1, i * 128 : (i + 1) * 128],
                                                rT[:, 4 * n + i : 4 * n + i + 1],
                                                ident[:, 0:128],
                                            )
                                        rs = ap.tile([1, 512], F32, tag="rs", bufs=2, name=f"rs{h}{n}")
                                        nc.vector.tensor_copy(rs[:], row[:])
                                        bc = aps.tile([64, 512], F32, tag="s", bufs=2, name=f"bc{h}{n}")
                                        nc.tensor.matmul(
                                            bc[:], ones[0:1, :], rs[0:1, :],
                                            start=True, stop=True,
                                        )
                                        tgt = (
                                            stage[:, n * 512 : (n + 1) * 512]
                                            if hh
                                            else att_sb[0:64, t, n * 512 : (n + 1) * 512]
                                        )
                                        nc.vector.tensor_mul(
                                            tgt, u_sb[:, n * 512 : (n + 1) * 512], bc[:]
                                        )
                                    if hh:
                                        nc.sync.dma_start(att_sb[64:128, t, :], stage[:])

                                # v projection (overlaps phase 0 on PE; av slots)
                                for m in range(NT):
                                    ps = aps.tile([128, 512], F32, tag="av", bufs=4, name=f"pv{m}")
                                    for j in range(8):
                                        nc.tensor.matmul(
                                            ps[:, 0:DSL],
                                            vt[j][:, m * 128 : (m + 1) * 128],
                                            wv_sb[:, j, :],
                                            start=(j == 0),
                                            stop=(j == 7),
                                        )
                                    nc.vector.tensor_copy(
                                        vh_sb[:, m, :].rearrange("p (h x) -> p h x", h=4)[
                                            :, :, 0:64
                                        ],
                                        ps[:, 0:DSL].rearrange("p (h x) -> p h x", h=4),
                                    )

                                for h in range(4):
                                    if h > 0:
                                        avs[h - 1] = [
                                            aps.tile([128, 512], F32, tag="av", bufs=4,
                                                     name=f"av{h - 1}{n}")
                                            for n in range(NC4)
                                        ]
                                    for m in range(NT):
                                        if h > 0:
                                            av_step(h - 1, m)
                                        s_step(h, m)
                                    if h > 0:
                                        ucopy(h - 1)
                                    if h > 1:
                                        normrest(h - 2)
                                avs[3] = [
                                    aps.tile([128, 512], F32, tag="av", bufs=4, name=f"av3{n}")
                                    for n in range(NC4)
                                ]
                                for m in range(NT):
                                    av_step(3, m)
                                ucopy(3)
                                normrest(2)
                                normrest(3)

                                # ---------------- output projection ----------------
                                for s in range(NT):
                                    for c in range(2):
                                        op = aps.tile([128, 512], F32, tag="av", bufs=4, name=f"op{s}{c}")
                                        for kt2 in range(2):
                                            nc.tensor.matmul(
                                                op[:],
                                                att_sb[:, kt2, s * 128 : (s + 1) * 128],
                                                wo_sb[:, kt2, c * 512 : (c + 1) * 512],
                                                start=(kt2 == 0),
                                                stop=(kt2 == 1),
                                            )
                                        ot = ap.tile([128, 512], F32, tag="o", bufs=4, name=f"ot{s}{c}")
                                        if (2 * s + c) % 2 == 0:
                                            nc.vector.tensor_copy(ot[:], op[:])
                                        else:
                                            nc.scalar.copy(ot[:], op[:])
                                        nc.sync.dma_start(
                                            out_d[s * 128 : (s + 1) * 128, c * 512 : (c + 1) * 512],
                                            ot[:],
                                        )
    nc.compile()
    names = dict(
        qT=qT_d.name, kT=kT_d.name, vT=vT_d.name,
        wq=wq_d.name, wk=wk_d.name, wv=wv_d.name, wo=wo_d.name, out=out_d.name,
    )
    return nc, names


def _dev_layout_x(x):
    # [seq, dm] f32 -> transposed [dm, seq] -> [128, 8, seq] bf16
    xt = np.ascontiguousarray(x.T).astype(BF16)
    return np.ascontiguousarray(xt.reshape(8, 128, SEQ).swapaxes(0, 1))


def _dev_layout_w(w):
    # [256, dm] slice -> W.T [dm, 256] -> [128, 8, 256] bf16
    wt = np.ascontiguousarray(w.T).astype(BF16)
    return np.ascontiguousarray(wt.reshape(8, 128, DSL).swapaxes(0, 1))


def kernel(q, k, v, W_q, b_q, W_k, b_k, W_v, b_v, W_o, b_o, trace=False):
    if "nc" not in _cache:
        _cache["nc"], _cache["names"] = _build()
    nc, names = _cache["nc"], _cache["names"]

    q, k, v = np.asarray(q), np.asarray(k), np.asarray(v)
    in_maps = []
    for core in range(8):
        b, g = core // 4, core % 4
        s256 = slice(256 * g, 256 * (g + 1))
        wo_slice = np.ascontiguousarray(np.asarray(W_o)[:, s256].T).astype(BF16)
        in_maps.append({
            names["qT"]: _dev_layout_x(q[b]),
            names["kT"]: _dev_layout_x(k[b]),
            names["vT"]: _dev_layout_x(v[b]),
            names["wq"]: _dev_layout_w(np.asarray(W_q)[s256] * 0.125),
            names["wk"]: _dev_layout_w(np.asarray(W_k)[s256]),
            names["wv"]: _dev_layout_w(np.asarray(W_v)[s256]),
            names["wo"]: np.ascontiguousarray(
                wo_slice.reshape(2, 128, DM).swapaxes(0, 1)
            ),
        })

    res = run_bass_kernel_spmd(nc, in_maps, core_ids=list(range(8)), trace=trace)
    out = np.zeros((2, SEQ, DM), np.float32)
    for core in range(8):
        out[core // 4] += res.results[core][names["out"]]
    out += np.asarray(b_o)[None, None, :].astype(np.float32)
    _cache["last_res"] = res
    return out
